# revision 6
# baseline (speedup 1.0000x reference)
"""Multi-head attention forward on 8 Trainium2 NeuronCores.

Problem: B=4, S=2048, E=1024, H=16, D=64 (fp32 in/out).

Sharding: 8 cores = (batch b, sequence half). Each core handles the full
key/value sequence of its batch (K/V projections computed redundantly by the
2 cores sharing a batch) and 1024 query rows, so outputs are disjoint and no
collective is needed. Inputs are host-rolled so each core's query rows are
rows 0:1024 of its x — softmax over keys is permutation invariant, so rolling
the key axis does not change the result.

x^T is pre-transposed on the HOST (numpy) and uploaded as [E, S] — plain
contiguous DMAs instead of the (slow, descriptor-storm) on-device DMA
transpose. All matmuls run in bf16 (fp32 PSUM accumulation).

The kernel is one software-pipelined stream built so the Scalar engine's exp
(~280us of ACTIVATE — the second roofline after the PE) runs concurrently
with ALL the PE work instead of only with the attention matmuls:

  Q^T proj for q-chunk 0, then per head pair j (16 iterations of (j, qc)):
    merge_i = [ scores MMs of iter i  interleaved kb-by-kb with
                ctx MMs of iter i-1 ] + exp ACTs of iter i
  K proj for pair j+1 and the V projection are woven between early merges;
  oproj for y rows 0:512 is woven into the qc=1 pass. The scores MM pair of
  the two heads (row groups 0:64 / 64:128) is emitted back-to-back so the PE
  can run them concurrently in disjoint row-group subarrays.
"""

import os
import sys
import types

import numpy as np

sys.path.insert(0, "/opt/trn_rl_repo")

B, S, E, H = 4, 2048, 1024, 16
D = E // H          # 64
Q = S // 2          # query rows per core
NCORES = 8

_compiled = None


def _install_prof_hook():
    try:
        import antenv.axon_hooks  # noqa: F401
        return
    except ImportError:
        pass
    try:
        import antenv
        from trn_agent_boot.trn_boot import _ntff_profile_via_ctypes
    except ImportError:
        return
    mod = types.ModuleType("antenv.axon_hooks")
    mod._hook = None
    mod.set_axon_ntff_profile_hook = lambda h: setattr(mod, "_hook", h)
    mod.get_axon_ntff_profile_hook = lambda: mod._hook
    sys.modules["antenv.axon_hooks"] = mod
    antenv.axon_hooks = mod
    try:
        mod._hook = _ntff_profile_via_ctypes("/opt/axon/libaxon_pjrt.so")
    except Exception:
        mod._hook = None


def _build():
    from contextlib import ExitStack

    from concourse import bacc
    import concourse.mybir as mybir
    from concourse import tile_utils
    from concourse.tile import TileContext

    tile_utils.max_sbuf_usage = 207 * 1024

    F32 = mybir.dt.float32
    BF16 = mybir.dt.bfloat16
    Exp = mybir.ActivationFunctionType.Exp

    nc = bacc.Bacc("TRN2", target_bir_lowering=False, debug=False)

    xt = nc.dram_tensor("xt", [E, S], BF16, kind="ExternalInput")   # x^T
    wq = nc.dram_tensor("wq", [E, E], BF16, kind="ExternalInput")
    wk = nc.dram_tensor("wk", [E, E], BF16, kind="ExternalInput")
    wv = nc.dram_tensor("wv", [E, E], BF16, kind="ExternalInput")
    wo = nc.dram_tensor("wo", [E, E], BF16, kind="ExternalInput")
    y = nc.dram_tensor("y", [Q, E], F32, kind="ExternalOutput")

    xt_v = xt.ap().rearrange("(eb p) s -> p eb s", p=128)           # [128, 8, 2048]
    wq_v = wq.ap().rearrange("(eb p) n -> p eb n", p=128)
    wk_v = wk.ap().rearrange("(eb p) n -> p eb n", p=128)
    wv_v = wv.ap().rearrange("(eb p) n -> p eb n", p=128)
    wo_v = wo.ap().rearrange("(eb p) n -> p eb n", p=128)
    y_v = y.ap().rearrange("(sb p) e -> sb p e", p=128)             # [8, 128, 1024]

    EB = E // 128        # 8 e-chunks
    SB = S // 128        # 16 s blocks (keys)
    KB = S // 128        # 16 key blocks

    with TileContext(nc) as tc:
        with ExitStack() as es:
            xtp = es.enter_context(tc.tile_pool(name="xt", bufs=1))
            kTp = es.enter_context(tc.tile_pool(name="kT", bufs=1))
            qTp = es.enter_context(tc.tile_pool(name="qT", bufs=1))
            vp = es.enter_context(tc.tile_pool(name="vA", bufs=1))
            ctxp = es.enter_context(tc.tile_pool(name="ctx", bufs=1))
            attnp = es.enter_context(tc.tile_pool(name="attn", bufs=6))
            wkqp = es.enter_context(tc.tile_pool(name="wkq", bufs=2))
            wvp = es.enter_context(tc.tile_pool(name="wvp", bufs=2))
            ytp = es.enter_context(tc.tile_pool(name="yt", bufs=2))
            nrmp = es.enter_context(tc.tile_pool(name="nrm", bufs=2))
            stgp = es.enter_context(tc.tile_pool(name="stg", bufs=2))
            psA = es.enter_context(tc.tile_pool(name="psA", bufs=2, space="PSUM"))
            psB = es.enter_context(tc.tile_pool(name="psB", bufs=4, space="PSUM"))

            xts = xtp.tile([128, EB, S], BF16)       # x^T  [e, s]
            kT = kTp.tile([128, EB, S], BF16)        # K^T  [n, s]
            qT = qTp.tile([128, EB, Q], BF16)        # Q^T  [n, q]
            vA = vp.tile([128, SB, H, D + 1], BF16)  # V | ones column
            ctx = ctxp.tile([128, EB, Q], BF16)      # ctx^T [e, q]

            # x^T arrives via 4 plain chunk DMAs (contiguous rows)
            for scc in range(4):
                nc.sync.dma_start(xts[:, :, scc * 512:(scc + 1) * 512],
                                  xt_v[:, :, scc * 512:(scc + 1) * 512])
            nc.gpsimd.memset(vA[:, :, :, D], 1.0)    # ones column (all heads)

            inv_sqrt_d = 1.0 / float(np.sqrt(D))

            def emit_qproj(qc):
                for nb in range(EB):
                    wt = wkqp.tile([128, EB, 128], BF16, tag="wkq",
                                   name=f"wtq{qc}_{nb}")
                    nc.scalar.dma_start(wt[:], wq_v[:, :, nb * 128:(nb + 1) * 128])
                    ps = psB.tile([128, 512], F32, tag="b", name=f"pq{qc}_{nb}")
                    for eb in range(EB):
                        nc.tensor.matmul(ps[:], wt[:, eb, :],
                                         xts[:, eb, qc * 512:(qc + 1) * 512],
                                         start=(eb == 0), stop=(eb == EB - 1))
                    nc.vector.tensor_copy(qT[:, nb, qc * 512:(qc + 1) * 512], ps[:])

            def emit_kproj(nb):
                wt = wkqp.tile([128, EB, 128], BF16, tag="wkq", name=f"wtk{nb}")
                nc.scalar.dma_start(wt[:], wk_v[:, :, nb * 128:(nb + 1) * 128])
                for sc in range(4):
                    ps = psB.tile([128, 512], F32, tag="b", name=f"pk{nb}_{sc}")
                    for eb in range(EB):
                        nc.tensor.matmul(ps[:], wt[:, eb, :],
                                         xts[:, eb, sc * 512:(sc + 1) * 512],
                                         start=(eb == 0), stop=(eb == EB - 1))
                    nc.vector.tensor_copy(kT[:, nb, sc * 512:(sc + 1) * 512], ps[:])

            def emit_vproj():
                for nc2 in range(2):
                    wvt = [None, None]
                    for ebh in range(2):
                        wvh = wvp.tile([128, 4, 512], BF16, tag="wv",
                                       name=f"wv{nc2}_{ebh}")
                        nc.scalar.dma_start(
                            wvh[:], wv_v[:, ebh * 4:(ebh + 1) * 4,
                                         nc2 * 512:(nc2 + 1) * 512])
                        wvt[ebh] = wvh
                    for sb in range(SB):
                        ps = psB.tile([128, 512], F32, tag="b",
                                      name=f"pv{nc2}_{sb}")
                        for eb in range(EB):
                            nc.tensor.matmul(
                                ps[:],
                                xts[:, eb, sb * 128:(sb + 1) * 128],
                                wvt[eb // 4][:, eb % 4, :],
                                start=(eb == 0), stop=(eb == EB - 1))
                        nc.vector.tensor_copy(
                            vA[:, sb, nc2 * 8:(nc2 + 1) * 8, 0:D],
                            ps.rearrange("p (h d) -> p h d", d=D))

            def emit_merge(cur, prev_state, extra=()):
                """Scores+exp for iteration `cur`, ctx MMs of `prev_state`
                interleaved kb-by-kb; `extra` emitters are run mid-merge.

                Returns (cur, attn_tiles, ctx_psum_pair)."""
                j, qc = cur
                qs = slice(qc * 512, (qc + 1) * 512)
                at = [[attnp.tile([128, 8, 512], BF16, tag="attn",
                                  name=f"at{j}_{qc}_{hh}_{i}")
                       for i in range(2)] for hh in range(2)]
                if prev_state is not None:
                    (pj, pqc), pat, pcps = prev_state
                extra = list(extra)
                for kbp in range(KB // 2):
                    sps = [psA.tile([128, 1024], F32, tag="sc",
                                    name=f"sc{j}_{qc}_{kbp}_{s}") for s in range(2)]
                    # scores: both heads' MMs back-to-back per (kbp, ki) so
                    # the PE can overlap them in disjoint row groups
                    for ki in range(2):
                        kb = 2 * kbp + ki
                        for hh in range(2):
                            p0 = hh * 64
                            nc.tensor.matmul(
                                sps[hh][:, ki * 512:(ki + 1) * 512],
                                kT[p0:p0 + 64, j, kb * 128:(kb + 1) * 128],
                                qT[p0:p0 + 64, j, qs],
                                start=True, stop=True)
                    for hh in range(2):
                        nc.scalar.activation(
                            at[hh][kbp // 4][:, (kbp % 4) * 2:(kbp % 4) * 2 + 2, :]
                            .rearrange("p a b -> p (a b)"),
                            sps[hh][:], Exp, scale=inv_sqrt_d)
                    # previous iteration's ctx, 2 kb per merge step
                    if prev_state is not None:
                        for ki in range(2):
                            kb = 2 * kbp + ki
                            for hh in range(2):
                                nc.tensor.matmul(
                                    pcps[hh][0:D + 1, :],
                                    vA[:, kb, 2 * pj + hh, :],
                                    pat[hh][kb // 8][:, kb % 8, :],
                                    start=(kb == 0), stop=(kb == KB - 1))
                    if extra and kbp in (3, 6):
                        extra.pop(0)()
                if prev_state is not None:
                    finish_ctx(prev_state)
                for fn in extra:
                    fn()
                cps = [psB.tile([128, 512], F32, tag="b",
                                name=f"cps{j}_{qc}_{i}") for i in range(2)]
                return (cur, at, cps)

            def finish_ctx(state):
                (pj, pqc), pat, pcps = state
                pqs = slice(pqc * 512, (pqc + 1) * 512)
                for hh in range(2):
                    cps = pcps[hh]
                    den = nrmp.tile([1, 512], F32, tag="den")
                    nc.vector.tensor_copy(den[:], cps[D:D + 1, :])
                    nc.vector.reciprocal_approx_fast(den[:], den[:])
                    bcast = nrmp.tile([64, 512], F32, tag="bc")
                    nc.gpsimd.partition_broadcast(bcast[:], den[:])
                    if hh == 0:
                        nc.vector.tensor_mul(
                            ctx[0:64, pj, pqs], cps[0:D, :], bcast[:])
                    else:
                        stg = stgp.tile([64, 512], BF16, tag="stg")
                        nc.vector.tensor_mul(stg[:], cps[0:D, :], bcast[:])
                        nc.sync.dma_start(ctx[64:128, pj, pqs], stg[:])

            def run_ctx_only(state):
                (pj, pqc), pat, pcps = state
                for kb in range(KB):
                    for hh in range(2):
                        nc.tensor.matmul(
                            pcps[hh][0:D + 1, :],
                            vA[:, kb, 2 * pj + hh, :],
                            pat[hh][kb // 8][:, kb % 8, :],
                            start=(kb == 0), stop=(kb == KB - 1))
                finish_ctx(state)

            wo_tiles = {}

            def load_wo(nc2):
                def fn():
                    wot = []
                    for ebh in range(2):
                        woh = wvp.tile([128, 4, 512], BF16, tag="wv",
                                       name=f"wo{nc2}_{ebh}")
                        nc.scalar.dma_start(
                            woh[:], wo_v[:, ebh * 4:(ebh + 1) * 4,
                                         nc2 * 512:(nc2 + 1) * 512])
                        wot.append(woh)
                    wo_tiles[nc2] = wot
                return fn

            def oproj_group(nc2, sb):
                def fn():
                    wot = wo_tiles[nc2]
                    ps = psB.tile([128, 512], F32, tag="b", name=f"yp{nc2}_{sb}")
                    for eb in range(EB):
                        nc.tensor.matmul(ps[:],
                                         ctx[:, eb, sb * 128:(sb + 1) * 128],
                                         wot[eb // 4][:, eb % 4, :],
                                         start=(eb == 0), stop=(eb == EB - 1))
                    yt = ytp.tile([128, 512], F32)
                    nc.vector.tensor_copy(yt[:], ps[:])
                    nc.sync.dma_start(y_v[sb][:, nc2 * 512:(nc2 + 1) * 512], yt[:])
                return fn

            # -------------------- the pipeline --------------------
            _sc = nc.named_scope("pipe"); _sc.__enter__()
            emit_qproj(0)
            emit_kproj(0)

            iters = [(j, 0) for j in range(H // 2)] + \
                    [(j, 1) for j in range(H // 2)]

            state = emit_merge(iters[0], None)          # scores(0,0)
            emit_kproj(1)
            emit_vproj()                                # V must precede ctx
            # merges 1..15: scores(i) + ctx(i-1), with K proj, Q second half
            # and first-half oproj woven in
            extras = {
                1: [lambda: emit_kproj(2)],
                2: [lambda: emit_kproj(3)],
                3: [lambda: emit_kproj(4)],
                4: [lambda: emit_kproj(5)],
                5: [lambda: emit_kproj(6)],
                6: [lambda: emit_kproj(7)],
                7: [lambda: emit_qproj(1)],
                9: [load_wo(0), oproj_group(0, 0)],
                10: [oproj_group(0, 1), oproj_group(0, 2)],
                11: [oproj_group(0, 3), load_wo(1)],
                12: [oproj_group(1, 0), oproj_group(1, 1)],
                13: [oproj_group(1, 2), oproj_group(1, 3)],
            }
            for i in range(1, len(iters)):
                state = emit_merge(iters[i], state, extras.get(i, ()))
            run_ctx_only(state)                         # ctx(7,1)

            # tail: y rows 512:1024 (wo nc2=1 still resident, then reload 0)
            for sb in range(4, 8):
                oproj_group(1, sb)()
            load_wo(0)()
            for sb in range(4, 8):
                oproj_group(0, sb)()
            _sc.__exit__(None, None, None)

    nc.compile()
    return nc


def kernel(x, Wq, Wk, Wv, Wo):
    global _compiled
    _install_prof_hook()
    import ml_dtypes
    from concourse import bass_utils

    if _compiled is None:
        _compiled = _build()
    nc = _compiled

    bf16 = ml_dtypes.bfloat16
    x = np.ascontiguousarray(x, dtype=np.float32)
    wq_b = np.ascontiguousarray(np.asarray(Wq, dtype=np.float32).astype(bf16))
    wk_b = np.ascontiguousarray(np.asarray(Wk, dtype=np.float32).astype(bf16))
    wv_b = np.ascontiguousarray(np.asarray(Wv, dtype=np.float32).astype(bf16))
    wo_b = np.ascontiguousarray(np.asarray(Wo, dtype=np.float32).astype(bf16))

    in_maps = []
    for c in range(NCORES):
        b, half = c // 2, c % 2
        xc = np.roll(x[b], -Q * half, axis=0) if half else x[b]
        in_maps.append({
            "xt": np.ascontiguousarray(xc.T.astype(bf16)),
            "wq": wq_b, "wk": wk_b, "wv": wv_b, "wo": wo_b,
        })

    trace = bool(int(os.environ.get("KERNEL_TRACE", "0")))
    res = bass_utils.run_bass_kernel_spmd(
        nc, in_maps, core_ids=list(range(NCORES)), trace=trace)
    kernel.last_result = res

    out = np.empty((B, S, E), dtype=np.float32)
    for c in range(NCORES):
        b, half = c // 2, c % 2
        out[b, half * Q:(half + 1) * Q] = res.results[c]["y"]
    return out


kernel.last_result = None
